# revision 1
# baseline (speedup 1.0000x reference)
"""CrossHeadAttention Trainium2 kernel (8-core SPMD, data+head parallel).

Reference computation (per batch b):
    k = x_enc @ Wk ; v = x_enc @ Wv ; q = x @ Wq        (bias-free linears)
    wei = softmax((q @ k^T) / sqrt(1024))  per head
    out = wei @ v                                        -> [B, T, H, D]

Sharding: 8 cores = 2 batches x 4 head-groups (4 heads each). Each core
receives x[b], x_enc[b] and the 256-column slice of Wq/Wk/Wv for its heads,
and produces out[b][:, :, hg*4:(hg+1)*4, :]. No cross-core communication.

Per-core dataflow (matmuls in float32r = full-rate ~fp32):
  x_enc --PE transpose--> xeT[c,s] --W-stationary matmul--> kT[d,s], vT[d,s]
  x     --PE transpose--> xT[c,t]  -----------------------> qT[d,t]
  vT --PE transpose--> v[s,d] (+ones column for softmax sums)
  S^T[s,t] = k q^T   (K=64 contraction, 2 heads row-packed via tile_position)
  P^T = exp(S^T / 32) on ScalarE (scores are ~N(0,1): no max-subtraction)
  outT[d_aug,t] = v_aug.T @ P^T  (psum-accumulated over s; row 64 = sums)
  out[t,d] = PE-transpose(outT) * 1/sums  (DVE), DMA to HBM.

The transposed activations are built in 512-column chunks that feed their
projections immediately and die, so SBUF holds one rotating 16 KiB/partition
chunk pool instead of 64 KiB static buffers. The kernel runs as two phases
with scoped PSUM pools: a projection phase (6-bank rotating psum; psum->sbuf
rounding copies split between DVE and the otherwise-idle ScalarE) and an
attention phase (4 banks score double-buffer + 2 PV accumulators + 2
finalize banks), with the exp activation table preloaded at t=0.
"""

from contextlib import ExitStack

import numpy as np

import concourse.bacc as bacc
import concourse.tile as tile
from concourse import mybir
from concourse.bass_utils import run_bass_kernel_spmd
from concourse.masks import make_identity

# Problem constants (hardcoded per spec)
B = 2
T = 2048          # query length
S = 2048          # key/value length
C = 1024          # n_embd
H = 16            # total heads
D = 64            # head size
N_CORES = 8
HG = H // (N_CORES // B)       # heads per core = 4
DCORE = HG * D                 # 256 projected dims per core
P = 128                        # partitions
CT = C // P                    # 8 contraction tiles
NPAIR = HG // 2                # 2 head pairs per core
TCH = 512                      # t-chunk width in attention
NTCH = T // TCH                # 4
ST = S // P                    # 16 s-tiles

F32 = mybir.dt.float32
F32R = mybir.dt.float32r
AF = mybir.ActivationFunctionType

SCALE = float(C) ** -0.5       # 1/32, folded into the exp activation


def _build_chain(nc, rows, aux, xtp, src_dram, projs, identity, rowtag):
    """Stream src[t, c] through PE-transpose into rotating [c, 512] chunks,
    and run every projection in `projs` on each chunk as soon as it lands.

    projs: list of (w_slice [P, CT, P] f32r, out_slice_fn(chunk_idx) -> AP).
    """
    for sch in range(src_dram.shape[0] // 512):
        _build_chain_chunk(nc, rows, aux, xtp, src_dram, projs, identity,
                           rowtag, sch, act_copies=True)


def _chain_chunk_pieces(nc, rows, aux, xtp, src_dram, projs, identity,
                        rowtag, sch, act_copies=False):
    """Emission pieces for one 512-wide x^T chunk + its projections.

    Returns a list of zero-arg callables; calling them in order (possibly
    interleaved with other emission) builds the chunk. When act_copies is
    set, half the psum->sbuf copies go to ScalarE instead of DVE (used
    pre-attention while ScalarE is otherwise idle).
    """
    state = {}

    def row_piece(r4):
        def go():
            if r4 == 0:
                state["xc"] = xtp.tile([P, CT, 512], F32R, tag="xch",
                                       name="xch")
            r = sch * 4 + r4
            row = rows.tile([P, C], F32, tag=rowtag, name="row")
            nc.sync.dma_start(out=row, in_=src_dram[r * P:(r + 1) * P, :])
            for cq in range(CT // 4):
                tp = aux.tile([P, 4 * P], F32, tag="aux", name="tp")
                for j in range(4):
                    ct = 4 * cq + j
                    nc.tensor.transpose(
                        tp[:, j * P:(j + 1) * P],
                        row[:, ct * P:(ct + 1) * P], identity)
                if act_copies and cq % 2:
                    copy_fn = nc.scalar.copy
                else:
                    copy_fn = lambda out, in_: nc.vector.tensor_copy(
                        out=out, in_=in_)
                copy_fn(
                    out=state["xc"][:, 4 * cq:4 * cq + 4,
                                    r4 * P:(r4 + 1) * P],
                    in_=tp.rearrange("p (j t) -> p j t", j=4))
        return go

    def proj_piece(w_slice, out_fn):
        def go():
            ps = aux.tile([P, 512], F32, tag="aux", name="ps")
            for ct in range(CT):
                nc.tensor.matmul(
                    ps, w_slice[:, ct, :], state["xc"][:, ct, :],
                    start=(ct == 0), stop=(ct == CT - 1))
            nc.vector.tensor_copy(out=out_fn(sch), in_=ps)
        return go

    return [row_piece(r4) for r4 in range(4)] +            [proj_piece(w, f) for w, f in projs]


def _build_chain_chunk(nc, rows, aux, xtp, src_dram, projs, identity,
                       rowtag, sch, act_copies=False):
    for piece in _chain_chunk_pieces(nc, rows, aux, xtp, src_dram, projs,
                                     identity, rowtag, sch, act_copies):
        piece()


def _build_v_transpose(nc, aux, vT, v_sb, identity, pt):
    """v_sb[s, 2pt:2pt+2, d] = (vT pair tile)^T via PE transpose."""
    for sq in range(ST // 4):
        tp = aux.tile([P, 4 * P], F32, tag="aux", name="tpv")
        for j in range(4):
            st = 4 * sq + j
            nc.tensor.transpose(
                tp[:, j * P:(j + 1) * P],
                vT.bitcast(F32)[:, st * P:(st + 1) * P], identity)
        for j in range(4):
            st = 4 * sq + j
            nc.vector.tensor_copy(
                out=v_sb[:, st, 2 * pt:2 * pt + 2, 0:D],
                in_=tp[:, j * P:(j + 1) * P].rearrange(
                    "p (h d) -> p h d", h=2))


def _build_attention_tch(nc, spsum, pvpools, aux, psb, otp, fin,
                         kT, qT, v_sb, identity, out, pair, tch,
                         interleave=()):
    """Attention st-loop for one head pair and one t-chunk -> oT tiles.

    `interleave`: emission pieces (e.g. next chunk's build) spliced between
    st iterations so the static schedule overlaps them with the exp stream.
    """
    if True:
        interleave = list(interleave)
        tsl = slice(tch * TCH, (tch + 1) * TCH)
        pv_ps = [pvpools[h2].tile([D + 1, TCH], F32, tag=f"pv{h2}",
                                  name=f"pv{h2}")
                 for h2 in range(2)]
        for st in range(ST):
            s_ps = spsum.tile([P, 2 * TCH], F32, tag="s", name="s_ps")
            for h2 in range(2):
                nc.tensor.matmul(
                    s_ps[:, h2 * TCH:(h2 + 1) * TCH],
                    kT[h2 * D:(h2 + 1) * D, pair, st * P:(st + 1) * P],
                    qT[h2 * D:(h2 + 1) * D, pair, tsl],
                    start=True, stop=True,
                    tile_position=(h2 * D, 0),
                )
            p_sb = psb.tile([P, 2 * TCH], F32R, tag="p", name="p_sb")
            nc.scalar.activation(out=p_sb, in_=s_ps, func=AF.Exp, scale=SCALE)
            for h2 in range(2):
                nc.tensor.matmul(
                    pv_ps[h2],
                    v_sb[:, st, 2 * pair + h2, :],
                    p_sb[:, h2 * TCH:(h2 + 1) * TCH],
                    start=(st == 0), stop=(st == ST - 1),
                )
            if interleave and st % 2 == 1:
                interleave.pop(0)()
        for piece in interleave:
            piece()
        oT = []
        for h2 in range(2):
            t_ = otp.tile([D + 1, TCH], F32, tag=f"oT{pair}{h2}",
                          name=f"oT{pair}{h2}")
            nc.vector.tensor_copy(out=t_, in_=pv_ps[h2])
            oT.append(t_)
        return oT


def _build_finalize_tch(nc, spsum, fin, oT, identity, out, pair, tch):
    """Transpose oT heads into a spsum bank, normalize by sums, store.

    Uses the spsum pool (not aux) so the next chunk-build's transposes are
    never serialized behind this tail work.
    """
    for sub in range(TCH // P):
        tt = tch * (TCH // P) + sub
        o_tile = fin.tile([P, 2 * D], F32, tag="o", name="o_tile")
        tp = spsum.tile([P, 2 * (D + 1)], F32, tag="ft", name="ft")
        for h2 in range(2):
            nc.tensor.transpose(
                tp[:, h2 * (D + 1):(h2 + 1) * (D + 1)],
                oT[h2][:, sub * P:(sub + 1) * P],
                identity[0:D + 1, 0:D + 1])
        tph = tp.rearrange("p (h e) -> p h e", h=2)
        r2 = fin.tile([P, 2], F32, tag="r", name="r2")
        nc.vector.reciprocal(out=r2, in_=tph[:, :, D])
        for h2 in range(2):
            nc.vector.tensor_scalar_mul(
                out=o_tile[:, h2 * D:(h2 + 1) * D],
                in0=tph[:, h2, 0:D], scalar1=r2[:, h2:h2 + 1])
        # SWDGE: keeps this dependent store out of SP's in-order
        # stream so it cannot head-of-line-block later row loads
        nc.gpsimd.dma_start(
            out=out[tt * P:(tt + 1) * P,
                    pair * 2 * D:(pair + 1) * 2 * D],
            in_=o_tile)


def _attention_phase(nc, tc, kT, qT, v_sb, identity, out,
                     psb, otp, fin):
    with tc.tile_pool(name="spsum", bufs=2, space="PSUM") as spsum, \
         tc.tile_pool(name="pvpsum0", bufs=1, space="PSUM") as pvp0, \
         tc.tile_pool(name="pvpsum1", bufs=1, space="PSUM") as pvp1, \
         tc.tile_pool(name="ftpsum", bufs=2, space="PSUM") as ftp:
        pvpools = (pvp0, pvp1)
        for tch in range(NTCH):
            oT0 = _build_attention_tch(
                nc, spsum, pvpools, None, psb, otp, fin,
                kT, qT, v_sb, identity, out, 0, tch)
            # pair-0 finalize emitted before pair-1 attention so its
            # transposes/stores run under pair-1's exp stream
            _build_finalize_tch(nc, ftp, fin, oT0, identity, out, 0, tch)
            oT1 = _build_attention_tch(
                nc, spsum, pvpools, None, psb, otp, fin,
                kT, qT, v_sb, identity, out, 1, tch)
            _build_finalize_tch(nc, ftp, fin, oT1, identity, out, 1, tch)


def _build_body(nc, tc, x, xe, wq, wk, wv, out):
    with ExitStack() as ctx:
        consts = ctx.enter_context(tc.tile_pool(name="consts", bufs=1))
        big = ctx.enter_context(tc.tile_pool(name="big", bufs=1))
        psb = ctx.enter_context(tc.tile_pool(name="psb", bufs=3))
        otp = ctx.enter_context(tc.tile_pool(name="otp", bufs=2))
        fin = ctx.enter_context(tc.tile_pool(name="fin", bufs=3))

        identity = consts.tile([P, P], F32)
        make_identity(nc, identity)
        # prime the ScalarE exp table at t=0 so the ~2.7us ACT_TABLE_LOAD is
        # off the critical path of the first real exp
        dummy = consts.tile([1, 2], F32)
        nc.vector.memset(dummy, 0.0)
        nc.scalar.activation(out=dummy, in_=dummy, func=AF.Exp)

        kT = big.tile([P, NPAIR, S], F32R, tag="kT")
        qT = big.tile([P, NPAIR, T], F32R, tag="qT")
        vT0 = big.tile([P, S], F32R, tag="vT0")
        vT1 = big.tile([P, S], F32R, tag="vT1")
        # v, with a ones column appended per head (col D) for softmax sums
        v_sb = big.tile([P, ST, HG, D + 1], F32R, tag="v_sb")
        nc.vector.memset(v_sb[:, :, :, D].bitcast(F32), 1.0)

        with tc.tile_pool(name="xtp", bufs=2) as xtp, \
             tc.tile_pool(name="rows", bufs=3) as rows, \
             tc.tile_pool(name="wpool", bufs=1) as wpool:

            # weights: DMA f32 staging -> DVE rounding copy -> f32r
            w_sbs = {}
            for name, wdram in (("wk", wk), ("wv", wv), ("wq", wq)):
                stage = wpool.tile([P, CT, DCORE], F32, tag="wstage",
                                   name="wstage")
                nc.gpsimd.dma_start(
                    out=stage, in_=wdram.rearrange("(ct p) d -> p ct d", p=P))
                wsb = wpool.tile([P, CT, DCORE], F32R, tag=f"{name}_sb",
                                 name=f"{name}_sb")
                nc.vector.tensor_copy(out=wsb, in_=stage)
                w_sbs[name] = wsb

            def _dsl(wname, dt_):
                return w_sbs[wname][:, :, dt_ * P:(dt_ + 1) * P]

            with tc.tile_pool(name="chainps", bufs=6, space="PSUM") as aux:
                # xe chain: k^T and v^T for both pairs, chunk-streamed
                _build_chain(
                    nc, rows, aux, xtp, xe,
                    [(_dsl("wk", 0),
                      lambda s: kT[:, 0, s * 512:(s + 1) * 512]),
                     (_dsl("wv", 0),
                      lambda s: vT0[:, s * 512:(s + 1) * 512]),
                     (_dsl("wk", 1),
                      lambda s: kT[:, 1, s * 512:(s + 1) * 512]),
                     (_dsl("wv", 1),
                      lambda s: vT1[:, s * 512:(s + 1) * 512])],
                    identity, "row")
                _build_v_transpose(nc, aux, vT0, v_sb, identity, 0)
                _build_v_transpose(nc, aux, vT1, v_sb, identity, 1)

                # x chain: q^T for both pairs
                qproj = [(_dsl("wq", 0),
                          lambda s: qT[:, 0, s * 512:(s + 1) * 512]),
                         (_dsl("wq", 1),
                          lambda s: qT[:, 1, s * 512:(s + 1) * 512])]
                _build_chain(nc, rows, aux, xtp, x, qproj, identity, "row")

            _attention_phase(nc, tc, kT, qT, v_sb, identity, out,
                             psb, otp, fin)


def build_program():
    nc = bacc.Bacc("TRN2", target_bir_lowering=False, debug=False,
                   num_devices=N_CORES)

    x = nc.dram_tensor("x", [T, C], F32, kind="ExternalInput").ap()
    xe = nc.dram_tensor("xe", [S, C], F32, kind="ExternalInput").ap()
    wq = nc.dram_tensor("wq", [C, DCORE], F32, kind="ExternalInput").ap()
    wk = nc.dram_tensor("wk", [C, DCORE], F32, kind="ExternalInput").ap()
    wv = nc.dram_tensor("wv", [C, DCORE], F32, kind="ExternalInput").ap()
    out = nc.dram_tensor("out", [T, DCORE], F32, kind="ExternalOutput").ap()

    with tile.TileContext(nc) as tc:
        _build_body(nc, tc, x, xe, wq, wk, wv, out)
    nc.compile()
    return nc


_NC_CACHE = None


def _get_program():
    global _NC_CACHE
    if _NC_CACHE is None:
        _NC_CACHE = build_program()
    return _NC_CACHE


def kernel(x_enc, x, Wk, Wq, Wv):
    x_enc = np.asarray(x_enc, dtype=np.float32)
    x = np.asarray(x, dtype=np.float32)
    Wk = np.asarray(Wk, dtype=np.float32)
    Wq = np.asarray(Wq, dtype=np.float32)
    Wv = np.asarray(Wv, dtype=np.float32)

    nc = _get_program()
    in_maps = []
    for core in range(N_CORES):
        b, hg = divmod(core, N_CORES // B)
        csl = slice(hg * DCORE, (hg + 1) * DCORE)
        in_maps.append({
            "x": np.ascontiguousarray(x[b]),
            "xe": np.ascontiguousarray(x_enc[b]),
            "wq": np.ascontiguousarray(Wq[:, csl]),
            "wk": np.ascontiguousarray(Wk[:, csl]),
            "wv": np.ascontiguousarray(Wv[:, csl]),
        })
    res = run_bass_kernel_spmd(nc, in_maps, list(range(N_CORES)))

    full = np.empty((B, T, H, D), dtype=np.float32)
    for core in range(N_CORES):
        b, hg = divmod(core, N_CORES // B)
        o = res.results[core]["out"].reshape(T, HG, D)
        full[b, :, hg * HG:(hg + 1) * HG, :] = o
    return full



# revision 2
# speedup vs baseline: 1.3585x; 1.3585x over previous
"""CrossHeadAttention Trainium2 kernel (8-core SPMD, data+head parallel).

Reference computation (per batch b):
    k = x_enc @ Wk ; v = x_enc @ Wv ; q = x @ Wq        (bias-free linears)
    wei = softmax((q @ k^T) / sqrt(1024))  per head
    out = wei @ v                                        -> [B, T, H, D]

Sharding: 8 cores = 2 batches x 4 head-groups (4 heads each). Each core
receives x[b], x_enc[b] and the 256-column slice of Wq/Wk/Wv for its heads,
and produces out[b][:, :, hg*4:(hg+1)*4, :]. No cross-core communication.

Per-core dataflow (bf16 matmuls; scores/exp in f32 psum):
  rows of x/x_enc (f32 DMA) -> bf16 convert (gpsimd) -> PE transpose with a
  bf16 identity (1 cyc/row) -> chunked x^T tiles that feed their projections
  immediately:
    kT[d,s], qT[d,t]  (W-stationary, x^T moving)
    v[s,d]            (x^T-stationary, W moving) + ones column for sums
  Attention is ACT-bound (all T*S*H exps run on ScalarE): it is organized as
  128 slots of one 1024-elem exp instruction each, covering
  (st-quarter sp, t-block tb, head-pair sw) groups of 4 s-tiles.
    scores^T[s,t] = kT @ qT  (two 512-wide bf16 matmuls into a 2-bank psum)
    p = exp(scores/32) -> bf16
    PV transposed: p[s,128t] stationary, v_aug[s,65] moving -> acc[t, 65]
  so the PV matmul streams only 65 columns/pass and the output lands in
  [t, d] orientation (no finalize transposes; sums ride in column 64).
  PV partial sums accumulate in two 1-bank psum tiles per group (one
  start=True per bank reuse, trailing writers rely on the 2KB zero-region
  pending-zero semantics) and are merged into an SBUF accumulator after each
  4-st group; the final quarter normalizes by the ones-column sums and DMAs
  [128,128] f32 slabs out. Projection pieces are interleaved into the slack
  of the exp-bound slots so the serial head stays ~10us.
"""

from contextlib import ExitStack

import numpy as np

import concourse.bacc as bacc
import concourse.tile as tile
from concourse import mybir
from concourse.bass_utils import run_bass_kernel_spmd
from concourse.masks import make_identity

# Problem constants (hardcoded per spec)
B = 2
T = 2048          # query length
S = 2048          # key/value length
C = 1024          # n_embd
H = 16            # total heads
D = 64            # head size
N_CORES = 8
HG = H // (N_CORES // B)       # heads per core = 4
DCORE = HG * D                 # 256 projected dims per core
P = 128                        # partitions
CT = C // P                    # 8 contraction tiles
NSW = 2                        # head-pair sweeps (dsl)
TB = 512                       # t-block width
NTB = T // TB                  # 4
ST = S // P                    # 16 s-tiles
NSP = 4                        # st quarters (passes)
STQ = ST // NSP                # 4 s-tiles per pass == s-tiles per xe chunk
NCH = 4                        # 512-row chunks per input tensor

F32 = mybir.dt.float32
BF16 = mybir.dt.bfloat16
AF = mybir.ActivationFunctionType

SCALE = float(C) ** -0.5       # 1/32, folded into the exp activation


def _build_body(nc, tc, x, xe, wq, wk, wv, out):
    with ExitStack() as ctx:
        consts = ctx.enter_context(tc.tile_pool(name="consts", bufs=1))
        big = ctx.enter_context(tc.tile_pool(name="big", bufs=1))
        rows = ctx.enter_context(tc.tile_pool(name="rows", bufs=3))
        brows = ctx.enter_context(tc.tile_pool(name="brows", bufs=3))
        xtp = ctx.enter_context(tc.tile_pool(name="xtp", bufs=2))
        ppool = ctx.enter_context(
            tc.tile_pool(name="ppool", bufs=2, space="PSUM"))
        psc = ctx.enter_context(tc.tile_pool(name="psc", bufs=2, space="PSUM"))
        pacc = ctx.enter_context(
            tc.tile_pool(name="pacc", bufs=2, space="PSUM"))
        ppool_sb = ctx.enter_context(tc.tile_pool(name="ppool_sb", bufs=3))
        fin = ctx.enter_context(tc.tile_pool(name="fin", bufs=4))
        wpool = ctx.enter_context(tc.tile_pool(name="wpool", bufs=1))

        identity = consts.tile([P, P], BF16)
        make_identity(nc, identity)
        # prime the ScalarE exp table at t=0 so the ACT_TABLE_LOAD is off the
        # critical path of the first real exp
        dummy = consts.tile([1, 2], F32)
        nc.vector.memset(dummy, 0.0)
        nc.scalar.activation(out=dummy, in_=dummy, func=AF.Exp)

        # persistent activation-derived tensors
        kT = big.tile([P, NSW, S], BF16, tag="kT")      # [2h'*64d, dsl, s]
        qT = big.tile([P, NSW, T], BF16, tag="qT")
        v_sb = big.tile([P, ST, HG, D + 1], BF16, tag="v_sb")
        nc.vector.memset(v_sb[:, :, :, D], 1.0)         # softmax-sum column
        acc_sb = big.tile([P, T // P, HG, D + 1], F32, tag="acc_sb")

        # weights: DMA f32 staging -> DVE rounding copy -> bf16
        w_sbs = {}
        for name, wdram in (("wk", wk), ("wq", wq), ("wv", wv)):
            stage = wpool.tile([P, CT, DCORE], F32, tag="wstage",
                               name="wstage")
            nc.gpsimd.dma_start(
                out=stage, in_=wdram.rearrange("(ct p) d -> p ct d", p=P))
            wsb = wpool.tile([P, CT, DCORE], BF16, tag=f"{name}_sb",
                             name=f"{name}_sb")
            nc.vector.tensor_copy(out=wsb, in_=stage)
            w_sbs[name] = wsb

        # ------------------------------------------------------------------
        # projection pieces (closures). Each piece is a zero-arg callable;
        # `state` carries the live x^T chunk tile per (src, chunk).
        # ------------------------------------------------------------------
        state = {}

        def row_piece(src_dram, key, sch, r4):
            def go():
                if r4 == 0:
                    state[key] = xtp.tile([P, CT, 512], BF16, tag="xch",
                                          name="xch")
                xt = state[key]
                r = sch * 4 + r4
                row = rows.tile([P, C], F32, tag="row", name="row")
                nc.sync.dma_start(out=row, in_=src_dram[r * P:(r + 1) * P, :])
                brow = brows.tile([P, C], BF16, tag="brow", name="brow")
                nc.gpsimd.tensor_copy(out=brow, in_=row)
                tp = ppool.tile([P, CT, P], BF16, tag="pp", name="tp")
                for ct in range(CT):
                    nc.tensor.transpose(
                        tp[:, ct, :], brow[:, ct * P:(ct + 1) * P], identity)
                nc.vector.tensor_copy(
                    out=xt[:, :, r4 * P:(r4 + 1) * P], in_=tp)
            return go

        def kq_piece(wname, dst, key, sch, dsl):
            def go():
                xt = state[key]
                ps = ppool.tile([P, 512], F32, tag="pp", name="ps")
                w = w_sbs[wname]
                for ct in range(CT):
                    nc.tensor.matmul(
                        ps, w[:, ct, dsl * P:(dsl + 1) * P], xt[:, ct, :],
                        start=(ct == 0), stop=(ct == CT - 1))
                nc.vector.tensor_copy(
                    out=dst[:, dsl, sch * 512:(sch + 1) * 512], in_=ps)
            return go

        def v_piece(key, sch, s4):
            def go():
                xt = state[key]
                ps = ppool.tile([P, DCORE], F32, tag="pp", name="psv")
                w = w_sbs["wv"]
                for ct in range(CT):
                    nc.tensor.matmul(
                        ps, xt[:, ct, s4 * P:(s4 + 1) * P], w[:, ct, :],
                        start=(ct == 0), stop=(ct == CT - 1))
                nc.vector.tensor_copy(
                    out=v_sb[:, sch * STQ + s4, :, 0:D],
                    in_=ps.rearrange("p (h d) -> p h d", h=HG))
            return go

        def xe_pieces(sch):
            key = ("xe", sch)
            return ([row_piece(xe, key, sch, r) for r in range(4)]
                    + [kq_piece("wk", kT, key, sch, d) for d in range(NSW)]
                    + [v_piece(key, sch, s4) for s4 in range(STQ)])

        def x_pieces(sch):
            key = ("x", sch)
            return ([row_piece(x, key, sch, r) for r in range(4)]
                    + [kq_piece("wq", qT, key, sch, d) for d in range(NSW)])

        # slot schedule: 128 slots; head pieces before slot 0, the rest
        # spread so chunk c is ready before the first group that needs it.
        head = xe_pieces(0) + x_pieces(0)
        slot_sched = {i: [] for i in range(NSP * NTB * NSW * STQ)}

        def spread(pieces, lo, hi):
            n = len(pieces)
            span = max(hi - lo, 1)
            for i, pc in enumerate(pieces):
                slot_sched[lo + (i * span) // n].append(pc)

        # x chunk tb is first needed at group (sp=0, tb, sw=0) = slot tb*8
        for tb in range(1, NTB):
            spread(x_pieces(tb), (tb - 1) * 8, tb * 8)
        # xe chunk sp is first needed at (sp, 0, 0) = slot sp*32
        for sp in range(1, NSP):
            spread(xe_pieces(sp), (sp - 1) * 32 + 2, sp * 32)

        for pc in head:
            pc()

        # ------------------------------------------------------------------
        # attention: 32 groups of 4 st-slots
        # ------------------------------------------------------------------
        slot = 0
        for sp in range(NSP):
            for tb in range(NTB):
                for sw in range(NSW):
                    accs = [pacc.tile([P, 2, 2, D + 1], F32, tag="acc",
                                      name=f"acc{a}") for a in range(2)]
                    first_pv = [True, True]
                    for st4 in range(STQ):
                        st = sp * STQ + st4
                        sc = psc.tile([P, 2, TB], F32, tag="sc", name="sc")
                        for h2 in range(2):
                            nc.tensor.matmul(
                                sc[:, h2, :],
                                kT[h2 * D:(h2 + 1) * D, sw,
                                   st * P:(st + 1) * P],
                                qT[h2 * D:(h2 + 1) * D, sw,
                                   tb * TB:(tb + 1) * TB],
                                start=True, stop=True)
                        p = ppool_sb.tile([P, 2, TB], BF16, tag="p", name="p")
                        nc.scalar.activation(out=p, in_=sc, func=AF.Exp,
                                             scale=SCALE)
                        last_st = st4 == STQ - 1
                        for tt in range(TB // P):
                            a = tt // 2
                            for h2 in range(2):
                                nc.tensor.matmul(
                                    accs[a][:, tt % 2, h2, :],
                                    p[:, h2, tt * P:(tt + 1) * P],
                                    v_sb[:, st, 2 * sw + h2, :],
                                    start=first_pv[a],
                                    stop=(last_st and tt % 2 == 1
                                          and h2 == 1),
                                    skip_group_check=True)
                                first_pv[a] = False
                        for pc in slot_sched[slot]:
                            pc()
                        slot += 1
                    # merge psum partials into the SBUF accumulator
                    for a in range(2):
                        dst = acc_sb[:, tb * 4 + 2 * a: tb * 4 + 2 * a + 2,
                                     2 * sw:2 * sw + 2, :]
                        if sp == 0:
                            nc.vector.tensor_copy(out=dst, in_=accs[a])
                        else:
                            nc.vector.tensor_add(dst, accs[a], dst)
                    if sp == NSP - 1:
                        _finalize(nc, fin, acc_sb, out, tb, sw)


def _finalize(nc, fin, acc_sb, out, tb, sw):
    """Normalize the two finished heads of t-block tb and store."""
    rcp = fin.tile([P, 4, 2], F32, tag="rcp", name="rcp")
    nc.vector.reciprocal(
        out=rcp, in_=acc_sb[:, tb * 4:tb * 4 + 4, 2 * sw:2 * sw + 2, D])
    for tt4 in range(4):
        ostage = fin.tile([P, 2 * D], F32, tag="ost", name="ostage")
        for h2 in range(2):
            nc.vector.tensor_scalar_mul(
                out=ostage[:, h2 * D:(h2 + 1) * D],
                in0=acc_sb[:, tb * 4 + tt4, 2 * sw + h2, 0:D],
                scalar1=rcp[:, tt4, h2:h2 + 1])
        tt = tb * 4 + tt4
        nc.sync.dma_start(
            out=out[tt * P:(tt + 1) * P, sw * 2 * D:(sw + 1) * 2 * D],
            in_=ostage)


def build_program():
    nc = bacc.Bacc("TRN2", target_bir_lowering=False, debug=False,
                   num_devices=N_CORES)

    x = nc.dram_tensor("x", [T, C], F32, kind="ExternalInput").ap()
    xe = nc.dram_tensor("xe", [S, C], F32, kind="ExternalInput").ap()
    wq = nc.dram_tensor("wq", [C, DCORE], F32, kind="ExternalInput").ap()
    wk = nc.dram_tensor("wk", [C, DCORE], F32, kind="ExternalInput").ap()
    wv = nc.dram_tensor("wv", [C, DCORE], F32, kind="ExternalInput").ap()
    out = nc.dram_tensor("out", [T, DCORE], F32, kind="ExternalOutput").ap()

    with tile.TileContext(nc) as tc:
        _build_body(nc, tc, x, xe, wq, wk, wv, out)
    nc.compile()
    return nc


_NC_CACHE = None


def _get_program():
    global _NC_CACHE
    if _NC_CACHE is None:
        _NC_CACHE = build_program()
    return _NC_CACHE


def kernel(x_enc, x, Wk, Wq, Wv):
    x_enc = np.asarray(x_enc, dtype=np.float32)
    x = np.asarray(x, dtype=np.float32)
    Wk = np.asarray(Wk, dtype=np.float32)
    Wq = np.asarray(Wq, dtype=np.float32)
    Wv = np.asarray(Wv, dtype=np.float32)

    nc = _get_program()
    in_maps = []
    for core in range(N_CORES):
        b, hg = divmod(core, N_CORES // B)
        csl = slice(hg * DCORE, (hg + 1) * DCORE)
        in_maps.append({
            "x": np.ascontiguousarray(x[b]),
            "xe": np.ascontiguousarray(x_enc[b]),
            "wq": np.ascontiguousarray(Wq[:, csl]),
            "wk": np.ascontiguousarray(Wk[:, csl]),
            "wv": np.ascontiguousarray(Wv[:, csl]),
        })
    res = run_bass_kernel_spmd(nc, in_maps, list(range(N_CORES)))

    full = np.empty((B, T, H, D), dtype=np.float32)
    for core in range(N_CORES):
        b, hg = divmod(core, N_CORES // B)
        o = res.results[core]["out"].reshape(T, HG, D)
        full[b, :, hg * HG:(hg + 1) * HG, :] = o
    return full


# revision 42
# speedup vs baseline: 1.4799x; 1.0893x over previous
"""CrossHeadAttention Trainium2 kernel (8-core SPMD, data+head parallel).

Reference computation (per batch b):
    k = x_enc @ Wk ; v = x_enc @ Wv ; q = x @ Wq        (bias-free linears)
    wei = softmax((q @ k^T) / sqrt(1024))  per head
    out = wei @ v                                        -> [B, T, H, D]

Sharding: 8 cores = 2 batches x 4 head-groups (4 heads each). Each core
receives x[b], x_enc[b] and the 256-column slice of Wq/Wk/Wv for its heads,
and produces out[b][:, :, hg*4:(hg+1)*4, :]. No cross-core communication.

The kernel is ACT-bound (all T*S*H/M = 16.7M exps run on ScalarE at 1
elem/cycle/lane: ~133us floor) with PE busy ~137us, so everything is
organized to keep the exp stream dense:

  Inputs arrive PRE-CONVERTED to bf16 by the host wrapper (device math is
  bf16 throughout, so the rounding just moves off-chip): input DMA halves
  and no on-device converts exist. x^T chunk tiles feed projections that
  die into persistent bf16 kT[d,dsl,s], qT[d,dsl,t], v[s,st,h,65] (ones
  column for softmax sums; all matmuls bf16 = 1 cyc/row, rel err 2.9e-3,
  bit-identical to on-device conversion). Transposition is split by zone:
  the latency-critical head chunks (x0, xe0, xe1) load as 128-row bf16
  slabs and go through the PE array (bf16 identity, 1 cyc/row) so the
  first exp waits only on wq + 4 rows + wk + 1 row (~15us); steady chunks
  (x1-x3, xe2, xe3) transpose straight out of DRAM on the DMA crossbar
  (dma_start_transpose, 14ns/16x128 tile) with zero engine work, keeping
  PE (~127us) under the ScalarE exp floor (~134us).

  Attention: 128 slots of one 1024-elem exp instruction (2 heads x 512 t),
  grouped by (st-range, t-block, head-pair):
    scores^T[s,t]: two 512-wide matmuls (kT slice stationary) into a
      2-of-4-bank psum ring
    p = exp(scores/32) -> bf16 (scale folded into the activation)
    PV transposed: p[s,128t] stationary, v_aug[s,65] moving -> acc[t,65],
      so PV streams only 65 cols/pass and the output lands [t, d] (no
      finalize transposes). PV batches trail the exp stream by one slot so
      a parked matmul burst never blocks the in-order PE sequencer ahead
      of the next scores.
  PV partials accumulate in two 1-bank psum tiles per group (single
  start=True per bank; later writers ride the 2KB zero-region pending-zero
  semantics with skip_group_check) and merge into an SBUF accumulator at
  group end. Three UNEVEN st-passes {chunks 0+1}, {2}, {3} defer the xe
  chunk builds to slots 64/96 where the PE is otherwise idle, since pass-0
  (the DMA-bound ramp) also has to absorb all four q-chunk builds.
  Finalize: reciprocal of the ones-column sums, per-partition scalar
  multiply, and two [128,2,128] f32 stores per (t-block, pair).

  Scheduling: projection work is emitted as "pieces" placed into specific
  exp slots (pieces must precede consumers in the per-engine instruction
  streams: Ldweights waits block the PE sequencer with no bypass). Row
  DMAs are split from transforms and prefetched; the first s-tiles of
  kT/v are projected per-128-column so each early slot only waits on its
  own just-landed row.
"""

from contextlib import ExitStack

import ml_dtypes
import numpy as np

import concourse.bacc as bacc
import concourse.tile as tile
from concourse import mybir
from concourse.bass_utils import run_bass_kernel_spmd
from concourse.masks import make_identity

# Problem constants (hardcoded per spec)
B = 2
T = 2048          # query length
S = 2048          # key/value length
C = 1024          # n_embd
H = 16            # total heads
D = 64            # head size
N_CORES = 8
HG = H // (N_CORES // B)       # heads per core = 4
DCORE = HG * D                 # 256 projected dims per core
P = 128                        # partitions
CT = C // P                    # 8 contraction tiles
NSW = 2                        # head-pair sweeps (dsl)
TB = 512                       # t-block width
NTB = T // TB                  # 4
ST = S // P                    # 16 s-tiles
NSP = 4                        # st quarters (passes)
STQ = ST // NSP                # 4 s-tiles per pass == s-tiles per xe chunk
NCH = 4                        # 512-row chunks per input tensor

F32 = mybir.dt.float32
BF16 = mybir.dt.bfloat16
AF = mybir.ActivationFunctionType

SCALE = float(C) ** -0.5       # 1/32, folded into the exp activation


def _build_body(nc, tc, x, xe, wq, wk, wv, out):
    with ExitStack() as ctx:
        consts = ctx.enter_context(tc.tile_pool(name="consts", bufs=1))
        big = ctx.enter_context(tc.tile_pool(name="big", bufs=1))
        rows = ctx.enter_context(tc.tile_pool(name="rows", bufs=6))
        xtp = ctx.enter_context(tc.tile_pool(name="xtp", bufs=5))
        ppool = ctx.enter_context(
            tc.tile_pool(name="ppool", bufs=2, space="PSUM"))
        psc = ctx.enter_context(tc.tile_pool(name="psc", bufs=2, space="PSUM"))
        pacc = ctx.enter_context(
            tc.tile_pool(name="pacc", bufs=2, space="PSUM"))
        ppool_sb = ctx.enter_context(tc.tile_pool(name="ppool_sb", bufs=4))
        fin = ctx.enter_context(tc.tile_pool(name="fin", bufs=4))
        wpool = ctx.enter_context(tc.tile_pool(name="wpool", bufs=1))

        identity = consts.tile([P, P], BF16)
        make_identity(nc, identity)
        # prime the ScalarE exp table at t=0 so the ACT_TABLE_LOAD is off the
        # critical path of the first real exp
        dummy = consts.tile([1, 2], F32)
        nc.vector.memset(dummy, 0.0)
        nc.scalar.activation(out=dummy, in_=dummy, func=AF.Exp)

        # persistent activation-derived tensors
        kT = big.tile([P, NSW, S], BF16, tag="kT")      # [2h'*64d, dsl, s]
        qT = big.tile([P, NSW, T], BF16, tag="qT")
        v_sb = big.tile([P, ST, HG, D + 1], BF16, tag="v_sb")
        nc.vector.memset(v_sb[:, :, :, D], 1.0)         # softmax-sum column
        acc_sb = big.tile([P, T // P, HG, D + 1], F32, tag="acc_sb")

        # weights: f32 staging via the sync queue (explicit DMA-device
        # ordering vs the critical x rows) -> gpsimd rounding copy -> bf16.
        # Loaded per 128-col head-pair half: only the dsl=0 halves sit on
        # the critical path to the first scores/PV.
        w_sbs = {}

        def load_w(name, wdram, dsl):
            def go():
                wsb = wpool.tile([P, CT, P], BF16, tag=f"{name}{dsl}_sb",
                                 bufs=1, name=f"{name}{dsl}_sb")
                nc.sync.dma_start(out=wsb, in_=wdram[dsl])
                w_sbs[(name, dsl)] = wsb
            return go

        # ------------------------------------------------------------------
        # projection pieces (closures). Row DMAs are split from the
        # convert/transpose work so loads can be prefetched several slots
        # ahead of the PE stream that consumes them (a not-yet-landed input
        # in the in-order PE stream stalls everything behind it).
        # `state` carries live tiles per (src, chunk).
        # ------------------------------------------------------------------
        state = {}

        def xbar_piece(src_dram, key, sch, cts):
            """Transpose 128-channel blocks of a bf16 input chunk straight
            into the x^T tile via the DMA crossbar (14ns/16x128 tile) —
            no engine work at all."""
            def go():
                if key not in state:
                    state[key] = xtp.tile([P, CT, 512], BF16, tag="xch",
                                          name="xch")
                xt = state[key]
                for ct in cts:
                    nc.sync.dma_start_transpose(
                        out=xt[:, ct, :],
                        in_=src_dram[sch * 512:(sch + 1) * 512,
                                     ct * P:(ct + 1) * P])
            return go

        def row_dma(src_dram, key, sch, r4):
            def go():
                row = rows.tile([P, C], BF16, tag="row", name="row")
                nc.sync.dma_start(
                    out=row, in_=src_dram[(sch * 4 + r4) * P:
                                          (sch * 4 + r4 + 1) * P, :])
                state[(key, r4)] = row
            return go

        def row_xf(key, r4):
            def go():
                if key not in state:
                    state[key] = xtp.tile([P, CT, 512], BF16, tag="xch",
                                          name="xch")
                xt = state[key]
                row = state.pop((key, r4))
                tp = ppool.tile([P, CT, P], BF16, tag="tp", bufs=1, name="tp")
                for ct in range(CT):
                    nc.tensor.transpose(
                        tp[:, ct, :], row[:, ct * P:(ct + 1) * P], identity)
                nc.vector.tensor_copy(
                    out=xt[:, :, r4 * P:(r4 + 1) * P], in_=tp)
            return go

        def kq_piece(wname, dst, key, sch, dsl, s4=None, act_copy=False,
                     part=None):
            """Project a chunk (or a single 128-col s-tile, which only
            needs one transposed row-group). act_copy routes the psum
            drain through the ScalarE, idle before the exp stream starts.
            part=0/1 emits the two 4-ct contraction halves as separate
            pieces so long PE bursts never sit ahead of a score matmul."""
            sl = (slice(0, 512) if s4 is None
                  else slice(s4 * P, (s4 + 1) * P))
            n = sl.stop - sl.start
            pskey = ("ps", wname, key, dsl, sl.start)

            def go():
                xt = state[key]
                if part in (None, 0):
                    ps = ppool.tile([P, n], F32, tag="pp", bufs=1, name="ps")
                    state[pskey] = ps
                else:
                    ps = state.pop(pskey)
                w = w_sbs[(wname, dsl)]
                cts = (range(CT) if part is None
                       else range(part * CT // 2, (part + 1) * CT // 2))
                for ct in cts:
                    nc.tensor.matmul(
                        ps, w[:, ct, :], xt[:, ct, sl],
                        start=(ct == 0), stop=(ct == CT - 1))
                if part in (None, 1):
                    copy = (nc.scalar.copy if act_copy
                            else nc.vector.tensor_copy)
                    copy(
                        out=dst[:, dsl,
                                sch * 512 + sl.start:sch * 512 + sl.stop],
                        in_=ps)
            return go

        def v_piece(key, sch, dsl, s4lo=0, s4hi=STQ // 2, act_copy=False,
                    part=None):
            """Project v for s-tiles [s4lo, s4hi) of a chunk into one psum
            bank (one start=True; later s-tiles rely on the 2KB zero-region
            pending-zero) and drain with a single strided copy. part=0/1
            splits the s-tile range into two emission pieces."""
            ns = s4hi - s4lo
            pskey = ("psv", key, dsl, s4lo)

            def go():
                xt = state[key]
                if part in (None, 0):
                    ps = ppool.tile([P, ns, P], F32, tag="pp", bufs=1,
                                    name="psv")
                    state[pskey] = ps
                else:
                    ps = state.pop(pskey)
                w = w_sbs[("wv", dsl)]
                idxs = (range(ns) if part is None
                        else range(part * ns // 2, (part + 1) * ns // 2))
                for i in idxs:
                    s4 = s4lo + i
                    for ct in range(CT):
                        nc.tensor.matmul(
                            ps[:, i, :], xt[:, ct, s4 * P:(s4 + 1) * P],
                            w[:, ct, :],
                            start=(i == 0 and ct == 0),
                            stop=(s4 == s4hi - 1 and ct == CT - 1),
                            skip_group_check=True)
                if part in (None, 1):
                    copy = (nc.scalar.copy if act_copy
                            else nc.vector.tensor_copy)
                    copy(
                        out=v_sb[:, sch * 4 + s4lo:sch * 4 + s4hi,
                                 2 * dsl:2 * dsl + 2, 0:D],
                        in_=ps.rearrange("p s (h d) -> p s h d", h=2))
            return go

        # slot schedule: 128 slots; head pieces before slot 0, the rest
        # spread so chunk c is ready before the first group that needs it.
        NSLOT = NSP * NTB * NSW * STQ
        slot_sched = {i: [] for i in range(NSLOT)}

        def spread(pieces, lo, hi):
            n = len(pieces)
            lo, hi = max(lo, 0), max(hi, 1)
            span = max(hi - lo, 1)
            for i, pc in enumerate(pieces):
                slot_sched[min(lo + (i * span) // n, NSLOT - 1)].append(pc)

        # head: bf16 row slabs (0.73us each) feed PE transposes directly --
        # no converts -- so the first exp only waits on wq + four x rows +
        # wk + one xe row; chunk-0/1 k and v are projected per-s-tile right
        # behind each arriving row. Steady chunks use the DMA crossbar.
        kx, kxe, kxe1 = ("x", 0), ("xe", 0), ("xe", 1)
        load_w("wq", wq, 0)()
        for r in range(4):
            row_dma(x, kx, 0, r)()
        load_w("wk", wk, 0)()
        row_dma(xe, kxe, 0, 0)()
        load_w("wv", wv, 0)()
        for r in (1, 2, 3):
            row_dma(xe, kxe, 0, r)()
        for r in range(4):
            row_dma(xe, kxe1, 1, r)()
        for r in range(4):
            row_xf(kx, r)()
        row_xf(kxe, 0)()
        kq_piece("wq", qT, kx, 0, 0)()
        kq_piece("wk", kT, kxe, 0, 0, s4=0)()
        v_piece(kxe, 0, 0, 0, 1)()

        slot_sched[0] += [load_w("wq", wq, 1), load_w("wk", wk, 1),
                          load_w("wv", wv, 1),
                          row_xf(kxe, 1),
                          kq_piece("wk", kT, kxe, 0, 0, s4=1),
                          v_piece(kxe, 0, 0, 1, 2)]
        slot_sched[1] += [row_xf(kxe, 2),
                          kq_piece("wk", kT, kxe, 0, 0, s4=2),
                          v_piece(kxe, 0, 0, 2, 3)]
        slot_sched[2] += [row_xf(kxe, 3),
                          kq_piece("wk", kT, kxe, 0, 0, s4=3),
                          v_piece(kxe, 0, 0, 3, 4)]
        slot_sched[3] += [row_xf(kxe1, 0),
                          kq_piece("wk", kT, kxe1, 1, 0, s4=0),
                          v_piece(kxe1, 1, 0, 0, 1)]
        slot_sched[4] += [row_xf(kxe1, 1),
                          kq_piece("wk", kT, kxe1, 1, 0, s4=1),
                          v_piece(kxe1, 1, 0, 1, 2)]
        slot_sched[5] += [row_xf(kxe1, 2),
                          kq_piece("wk", kT, kxe1, 1, 0, s4=2),
                          v_piece(kxe1, 1, 0, 2, 3),
                          kq_piece("wk", kT, kxe, 0, 1, part=0)]
        slot_sched[6] += [row_xf(kxe1, 3),
                          kq_piece("wk", kT, kxe1, 1, 0, s4=3),
                          v_piece(kxe1, 1, 0, 3, 4),
                          kq_piece("wk", kT, kxe, 0, 1, part=1)]
        slot_sched[7] += [kq_piece("wq", qT, kx, 0, 1),
                          v_piece(kxe, 0, 1, part=0)]
        slot_sched[8] += [v_piece(kxe, 0, 1, part=1),
                          kq_piece("wk", kT, kxe1, 1, 1, part=0)]
        slot_sched[9] += [kq_piece("wk", kT, kxe1, 1, 1, part=1),
                          v_piece(kxe1, 1, 1, part=0)]
        slot_sched[10] += [v_piece(kxe1, 1, 1, part=1)]

        # steady chunks via the crossbar: q(x-chunk tb) first used at slot
        # tb*16; the pass-1/2 xe chunks at slots 64 / 96.
        for tb, use in ((1, 16), (2, 32), (3, 48)):
            key = ("x", tb)
            spread([xbar_piece(x, key, tb, range(CT))], use - 12, use - 11)
            spread([kq_piece("wq", qT, key, tb, d, part=pt)
                    for d in range(NSW) for pt in (0, 1)],
                   use - 8, use - 1)
        for c, use in ((2, 64), (3, 96)):
            key = ("xe", c)
            spread([xbar_piece(xe, key, c, range(CT))], use - 14, use - 13)
            spread([kq_piece("wk", kT, key, c, 0, part=0),
                    kq_piece("wk", kT, key, c, 0, part=1),
                    v_piece(key, c, 0, part=0),
                    v_piece(key, c, 0, part=1),
                    kq_piece("wk", kT, key, c, 1, part=0),
                    kq_piece("wk", kT, key, c, 1, part=1),
                    v_piece(key, c, 1, part=0),
                    v_piece(key, c, 1, part=1)],
                   use - 12, use - 1)

        # ------------------------------------------------------------------
        # attention: passes over uneven st ranges. Pass 0 covers xe chunks
        # 0-1 (built during the DMA-bound ramp); chunks 2 and 3 are only
        # pulled in at slots 64 / 96, so their projection pieces land in the
        # otherwise ACT-bound (PE-idle) second half.
        # ------------------------------------------------------------------
        slot = 0
        passes = [(0, 8), (8, 12), (12, 16)]
        glist = [(lo, hi, tb, sw) for (lo, hi) in passes
                 for tb in range(NTB) for sw in range(NSW)]
        lastv = {}
        seen = set()
        for gi, (lo, hi, tb, sw) in enumerate(glist):
            lastv[(tb, sw)] = gi
        for gi, (lo, hi, tb, sw) in enumerate(glist):
            accs = [pacc.tile([P, 2, 2, D + 1], F32, tag="acc",
                              name=f"acc{a}") for a in range(2)]
            first_pv = [True, True]

            def do_merge(a):
                dst = acc_sb[:, tb * 4 + 2 * a: tb * 4 + 2 * a + 2,
                             2 * sw:2 * sw + 2, :]
                if (tb, sw) not in seen:
                    nc.vector.tensor_copy(out=dst, in_=accs[a])
                else:
                    nc.vector.tensor_add(dst, accs[a], dst)

            def pv_batch(st, tail=False):
                pt = pend.pop(0)
                for tt in range(TB // P):
                    a = tt // 2
                    for h2 in range(2):
                        nc.tensor.matmul(
                            accs[a][:, tt % 2, h2, :],
                            pt[:, h2, tt * P:(tt + 1) * P],
                            v_sb[:, st, 2 * sw + h2, :],
                            start=first_pv[a],
                            stop=(st == hi - 1 and tt % 2 == 1
                                  and h2 == 1),
                            skip_group_check=True)
                        first_pv[a] = False
                    # on the very last batch, merge each accumulator the
                    # moment its final PV is in, shortening the tail chain
                    if tail and tt % 2 == 1:
                        do_merge(tt // 2)

            pend = []
            for st in range(lo, hi):
                # In the chunk-0/1 era, pieces PRODUCE the kT/qT/v this
                # very slot consumes, so they must precede it in the
                # in-order engine streams. In steady state pieces feed
                # later slots only and are emitted between the exp and the
                # trailing PV batch, so a late piece or a parked PV burst
                # never gates the next exp.
                if slot < 11:
                    for pc in slot_sched[slot]:
                        pc()
                sc = psc.tile([P, 2, TB], F32, tag="sc", name="sc")
                for h2 in range(2):
                    nc.tensor.matmul(
                        sc[:, h2, :],
                        kT[h2 * D:(h2 + 1) * D, sw, st * P:(st + 1) * P],
                        qT[h2 * D:(h2 + 1) * D, sw, tb * TB:(tb + 1) * TB],
                        start=True, stop=True)
                p = ppool_sb.tile([P, 2, TB], BF16, tag="p", name="p")
                nc.scalar.activation(out=p, in_=sc, func=AF.Exp,
                                     scale=SCALE)
                pend.append(p)
                if slot >= 11:
                    for pc in slot_sched[slot]:
                        pc()
                # PV batches trail one slot behind the exp stream
                if st > lo:
                    pv_batch(st - 1)
                if st == hi - 1:
                    pv_batch(st, tail=(gi == len(glist) - 1))
                slot += 1
            # merge psum partials into the SBUF accumulator
            if gi != len(glist) - 1:
                for a in range(2):
                    do_merge(a)
            seen.add((tb, sw))
            if lastv[(tb, sw)] == gi:
                _finalize(nc, fin, acc_sb, out, tb, sw)


def _finalize(nc, fin, acc_sb, out, tb, sw, use_act=False):
    """Normalize the two finished heads of t-block tb and store. Two DMAs
    (2 t-tiles each) so the second store's DGE setup hides under the
    first's transfer; the last group's muls run on the idle ScalarE."""
    rcp = fin.tile([P, 4, 2], F32, tag="rcp", name="rcp")
    nc.vector.reciprocal(
        out=rcp, in_=acc_sb[:, tb * 4:tb * 4 + 4, 2 * sw:2 * sw + 2, D])
    for half in range(2):
        ostage = fin.tile([P, 2, 2 * D], F32, tag="ost", name="ostage")
        for i in range(2):
            tt4 = half * 2 + i
            for h2 in range(2):
                o = ostage[:, i, h2 * D:(h2 + 1) * D]
                a = acc_sb[:, tb * 4 + tt4, 2 * sw + h2, 0:D]
                r = rcp[:, tt4, h2:h2 + 1]
                if use_act:
                    nc.scalar.activation(
                        out=o, in_=a, func=mybir.ActivationFunctionType.Copy,
                        scale=r)
                else:
                    nc.vector.tensor_scalar_mul(out=o, in0=a, scalar1=r)
        t0 = (tb * 4 + half * 2) * P
        nc.sync.dma_start(
            out=out[t0:t0 + 2 * P,
                    sw * 2 * D:(sw + 1) * 2 * D].rearrange(
                        "(tt p) c -> p tt c", p=P),
            in_=ostage)


def build_program():
    nc = bacc.Bacc("TRN2", target_bir_lowering=False, debug=False,
                   num_devices=N_CORES)

    # Inputs arrive pre-converted to bf16 by the host wrapper (the device
    # math is bf16 throughout, so this only moves the rounding off-chip):
    # halves the input DMA and lets every transpose run on the DMA crossbar
    # straight out of DRAM.
    x = nc.dram_tensor("x", [T, C], BF16, kind="ExternalInput").ap()
    xe = nc.dram_tensor("xe", [S, C], BF16, kind="ExternalInput").ap()
    wq = nc.dram_tensor("wq", [NSW, P, CT, P], BF16,
                        kind="ExternalInput").ap()
    wk = nc.dram_tensor("wk", [NSW, P, CT, P], BF16,
                        kind="ExternalInput").ap()
    wv = nc.dram_tensor("wv", [NSW, P, CT, P], BF16,
                        kind="ExternalInput").ap()
    out = nc.dram_tensor("out", [T, DCORE], F32, kind="ExternalOutput").ap()

    with tile.TileContext(nc) as tc:
        _build_body(nc, tc, x, xe, wq, wk, wv, out)
    nc.compile()
    return nc


_NC_CACHE = None


def _get_program():
    global _NC_CACHE
    if _NC_CACHE is None:
        _NC_CACHE = build_program()
    return _NC_CACHE


def _wlayout(w):
    """[1024, 256] f32 -> [dsl, p, ct, d] bf16, contiguous per 128-col half
    so each half loads in one penalty-free DMA."""
    w = w.reshape(CT, P, NSW, P).transpose(2, 1, 0, 3)
    return np.ascontiguousarray(w).astype(ml_dtypes.bfloat16)


def kernel(x_enc, x, Wk, Wq, Wv):
    bf16 = ml_dtypes.bfloat16
    x_enc = np.asarray(x_enc, dtype=np.float32)
    x = np.asarray(x, dtype=np.float32)
    Wk = np.asarray(Wk, dtype=np.float32)
    Wq = np.asarray(Wq, dtype=np.float32)
    Wv = np.asarray(Wv, dtype=np.float32)

    nc = _get_program()
    in_maps = []
    for core in range(N_CORES):
        b, hg = divmod(core, N_CORES // B)
        csl = slice(hg * DCORE, (hg + 1) * DCORE)
        in_maps.append({
            "x": np.ascontiguousarray(x[b]).astype(bf16),
            "xe": np.ascontiguousarray(x_enc[b]).astype(bf16),
            "wq": _wlayout(Wq[:, csl]),
            "wk": _wlayout(Wk[:, csl]),
            "wv": _wlayout(Wv[:, csl]),
        })
    res = run_bass_kernel_spmd(nc, in_maps, list(range(N_CORES)))

    full = np.empty((B, T, H, D), dtype=np.float32)
    for core in range(N_CORES):
        b, hg = divmod(core, N_CORES // B)
        o = res.results[core]["out"].reshape(T, HG, D)
        full[b, :, hg * HG:(hg + 1) * HG, :] = o
    return full
